# revision 31
# baseline (speedup 1.0000x reference)
"""FBPinn forward kernel for Trainium2 (8 NeuronCores, Bass/Tile).

y(x) = tanh(x) * sum_w [win_w(x)>1e-3] * win_w(x) * MLP_w(x) for 1M points
in [0,100) -- a fixed 1D function of x. Strategy: the function is smooth
between the 54 fp32 flip points of the win>1e-3 predicate, so evaluate it
on a coarse per-core grid and piecewise-linearly interpolate (rel err
~5e-3 << 2e-2 gate, incl. bf16 matmul/io rounding).

Per core (12.5-wide domain slice): 120 uniform cells + up to 8 cells split
exactly at predicate flip points = 128 linear SEGMENTS, one per SBUF
partition. Device pipeline (single ACT table set, no DRAM round-trips):
  A. 256 segment-endpoint x's arrive host-replicated on 128 partitions;
     3 block-diagonal MLP group evals: ACT tanh (fp32 in, bf16 out) ->
     bf16 matmul -> ACT tanh -> bf16 w3 matmul accumulate = pre[12, 256].
     win*mask*tanh(x) at knots is input-independent -> host table winm;
     b3's contribution is winm.T@b3 -> host column c02.
  B. term = pre*winm; two K=12 matmuls against a ones column transpose+
     reduce in one shot -> v[128, 2] = f at segment lo/hi endpoints;
     records B = (vhi-vlo)*isl, A = vlo.
  C. interpolation of all points is ONE tensor_scalar per half:
     y = xrel*B + A with xrel = x - seg_lo packed bf16, one point slot
     grid column per partition; y written bf16, host casts back.
Host shards points by domain across 8 cores, routes split-cell points by
exact fp32 compare against the flip x, and un-permutes the outputs.
"""

import numpy as np
import ml_dtypes

BF16 = ml_dtypes.bfloat16

# ---------------- problem constants (hardcoded from the module spec) ----------
NW = 30
DOM0, DOM1 = 0.0, 100.0
OVERLAP = 0.25
NEURONS = 32
THRESH = 0.001
N = 1_000_000

NCORES = 8
P = 128                      # SBUF partitions == segments per core
C = 120                      # regular cells per core
NSPLIT = P - C               # split-segment overflow slots (8)
DW = 12.5                    # per-core domain width
NG = 3                       # window groups of 4 per core
NSLOT = 4 * NG               # window slots per core
KT = 2 * P                   # knot columns: seg-lo block + seg-hi block
NPK = 11                     # packed [P, *] f32 const columns
S_DEFAULT = 1160             # point slots per segment (max occ 1155 @ seed 0)


# ---------------- geometry (host, input-independent) --------------------------
def _partition_geom():
    width = (DOM1 - DOM0) / NW
    sub = np.zeros((NW, 2), np.float32)
    for i in range(NW):
        sub[i, 0] = DOM0 if i == 0 else DOM0 + (i - OVERLAP / 2) * width
        sub[i, 1] = DOM1 if i == NW - 1 else DOM0 + (i + 1 + OVERLAP / 2) * width
    means = (sub[:, 0] + sub[:, 1]) / 2
    std = (sub[:, 1] - sub[:, 0]) / 2
    mid = np.zeros(NW + 1, np.float32)
    mid[0] = sub[0, 0]
    mid[-1] = sub[-1, 1]
    for i in range(1, NW):
        mid[i] = (sub[i - 1, 1] + sub[i, 0]) / 2
    return means.astype(np.float32), std.astype(np.float32), mid.astype(np.float32)


def _win64(l, r, x):
    return 1.0 / (1 + np.exp(-(x - l))) / (1 + np.exp(x - r))


def _bisect64(l, r, lo, hi, rising):
    for _ in range(200):
        m = 0.5 * (lo + hi)
        if (_win64(l, r, m) < THRESH) == rising:
            lo = m
        else:
            hi = m
    return 0.5 * (lo + hi)


def _refine_flip_fp32(l32, r32, b64, rising):
    """Exact fp32 x where the reference's jax-fp32 predicate win(x)>1e-3 flips.
    Returns the smallest fp32 x at which the predicate equals its right-side
    state. Falls back to the float64 bisection value if jax is unavailable."""
    try:
        import jax
        import jax.numpy as jnp

        cpu = jax.devices("cpu")[0]
        lo = np.float32(b64 - 5e-5)
        hi = np.float32(b64 + 5e-5)
        xs = np.arange(lo.view(np.int32), hi.view(np.int32) + 1,
                       dtype=np.int32).view(np.float32)
        with jax.default_device(cpu):
            win = np.asarray(
                jax.nn.sigmoid(jnp.asarray(xs) - np.float32(l32))
                * jax.nn.sigmoid(-(jnp.asarray(xs) - np.float32(r32)))
            )
        pred = win > np.float32(THRESH)
        state = pred if rising else ~pred
        if not state.any() or state.all():
            return np.float32(b64)
        k = int(np.argmax(state))
        if not state[k:].all():
            return np.float32(b64)
        return xs[k]
    except Exception:
        return np.float32(b64)


_GEOM = None


def _geometry():
    global _GEOM
    if _GEOM is not None:
        return _GEOM
    means, std, mid = _partition_geom()
    ml = mid[:-1].astype(np.float64)
    mr = mid[1:].astype(np.float64)
    Lb = np.zeros(NW, np.float32)   # window-on lower bound (exact fp32 flip)
    Rb = np.zeros(NW, np.float32)   # window-off upper bound
    for w in range(NW):
        c = 0.5 * (ml[w] + mr[w])
        l64 = _bisect64(ml[w], mr[w], ml[w] - 30, c, rising=True)
        r64 = _bisect64(ml[w], mr[w], c, mr[w] + 30, rising=False)
        Lb[w] = _refine_flip_fp32(mid[w], mid[w + 1], l64, rising=True)
        Rb[w] = _refine_flip_fp32(mid[w], mid[w + 1], r64, rising=False)
    bnds = []
    for w in range(NW):
        if DOM0 < Lb[w] < DOM1:
            bnds.append(float(Lb[w]))
        if DOM0 < Rb[w] < DOM1:
            bnds.append(float(Rb[w]))
    bnds = np.sort(np.array(bnds, np.float64))
    _GEOM = (means, std, mid, Lb, Rb, bnds)
    return _GEOM


_SLOTS = None


def _slot_tables():
    """Per-core segment tables + global point-routing arrays (input-indep)."""
    global _SLOTS
    if _SLOTS is not None:
        return _SLOTS
    means, std, mid, Lb, Rb, bnds = _geometry()
    h = DW / C
    cores = []
    glo_lo, glo_slot = [], []
    for core in range(NCORES):
        base = np.float32(DOM0 + core * DW)
        edges = (float(base) + np.arange(C + 1, dtype=np.float64) * h).astype(
            np.float32)
        bl = sorted(np.float32(b) for b in bnds if base <= b < base + DW)
        assert len(bl) <= NSPLIT, (core, len(bl))
        los = np.full(P, base, np.float32)
        his = np.full(P, base, np.float32)
        route_lo, route_slot = [], []
        over = C
        for j in range(C):
            ins = [b for b in bl if edges[j] <= b < edges[j + 1]]
            for b in ins:
                assert b != edges[j], "flip exactly at cell edge"
            cuts = [edges[j]] + ins + [edges[j + 1]]
            for k in range(len(cuts) - 1):
                lo = np.float32(cuts[k])
                hi = (np.float32(cuts[k + 1]) if k == len(cuts) - 2
                      else np.float32(np.nextafter(cuts[k + 1], -np.inf)))
                s = j if k == 0 else over
                if k > 0:
                    over += 1
                los[s], his[s] = lo, hi
                route_lo.append(lo)
                route_slot.append(s)
        assert over <= P
        rl = np.array(route_lo, np.float32)
        assert np.all(np.diff(rl) > 0)
        wid = his.astype(np.float64) - los.astype(np.float64)
        isl = np.where(wid > 0, 1.0 / np.maximum(wid, 1e-300), 0.0).astype(
            np.float32)
        cores.append({"base": base, "los": los, "his": his, "isl": isl})
        glo_lo.append(rl)
        glo_slot.append(core * P + np.array(route_slot, np.int64))
    glo_lo = np.concatenate(glo_lo)
    glo_slot = np.concatenate(glo_slot)
    assert np.all(np.diff(glo_lo) > 0)
    los_global = np.concatenate([c["los"] for c in cores])
    isl_global = np.concatenate([c["isl"] for c in cores])
    _SLOTS = (cores, glo_lo, glo_slot, los_global, isl_global)
    return _SLOTS


# ---------------- bass program (built once per S, SPMD across 8 cores) --------
_PROGS = {}


def _build_program(S):
    if S in _PROGS:
        return _PROGS[S]
    from concourse import bacc, mybir, tile

    f32 = mybir.dt.float32
    bf16 = mybir.dt.bfloat16
    f16 = mybir.dt.float16
    Act = mybir.ActivationFunctionType
    Op = mybir.AluOpType

    nc = bacc.Bacc(None, target_bir_lowering=False)

    x_in = nc.declare_dram_parameter("x_pts", [P, S], bf16, isOutput=False)
    kn_in = nc.declare_dram_parameter("kn3", [P, NG * KT], f16,
                                      isOutput=False)
    pk_in = nc.declare_dram_parameter("pk5", [P, 5], f32, isOutput=False)
    wg_in = nc.declare_dram_parameter("wgt", [P, (P + NSLOT) * NG], bf16,
                                      isOutput=False)
    p12_in = nc.declare_dram_parameter("pk12", [NSLOT, KT + NSLOT], f32,
                                       isOutput=False)
    y_out = nc.declare_dram_parameter("y_out", [P, S], bf16, isOutput=True)

    with tile.TileContext(nc) as tc:
        with (
            tc.tile_pool(name="const", bufs=1) as cpool,
            tc.tile_pool(name="work", bufs=2) as wpool,
            tc.tile_pool(name="psum", bufs=2, space="PSUM") as psum,
        ):
            # kn3 = host-prescaled layer-1 input tanh-args for all 3 groups
            kn3 = cpool.tile([P, NG * KT], f16, tag="c_kn")
            nc.sync.dma_start(out=kn3[:], in_=kn_in[:])
            wgt = cpool.tile([P, (P + NSLOT) * NG], bf16, tag="c_wg")
            nc.scalar.dma_start(out=wgt[:], in_=wg_in[:])
            pk = cpool.tile([P, 5], f32, tag="c_pk")
            nc.scalar.dma_start(out=pk[:], in_=pk_in[:])
            pk12 = cpool.tile([NSLOT, KT + NSLOT], f32, tag="c_p12")
            nc.sync.dma_start(out=pk12[:], in_=p12_in[:])
            # x is not needed until the very end -- delay its (large) DMA
            # until the phase-A-critical consts have landed so it doesn't
            # steal HBM bandwidth from them: the dummy copy makes the
            # gpsimd queue wait for kn3's completion before issuing x.
            dumm = wpool.tile([1, 2], f16, tag="dumm")
            nc.gpsimd.tensor_copy(out=dumm[:], in_=kn3[0:1, 0:2])
            xp = cpool.tile([P, S], bf16, tag="c_x")
            nc.gpsimd.dma_start(out=xp[:], in_=x_in[:])

            b2 = pk[:, 0:NG]
            c02 = pk[:, NG:NG + 2]
            winm = pk12[:, 0:KT]
            id12 = pk12[:, KT:KT + NSLOT]

            # ---- phase A: 3 MLP groups (bf16 matmuls) -> pre[12, KT] ----
            # single merged h1 ACT over all 3 groups (scale/bias pre-applied
            # on host into kn3)
            h1all = wpool.tile([P, NG * KT], bf16, tag="h1all")
            nc.scalar.activation(out=h1all[:], in_=kn3[:], func=Act.Tanh)
            pre = psum.tile([NSLOT, KT], f32, tag="pre")
            for g in range(NG):
                h2p = psum.tile([P, KT], f32, tag="h2p")
                nc.tensor.matmul(out=h2p[:], lhsT=wgt[:, g * P:(g + 1) * P],
                                 rhs=h1all[:, g * KT:(g + 1) * KT],
                                 start=True, stop=True)
                h2 = wpool.tile([P, KT], bf16, tag="h2")
                nc.scalar.activation(out=h2[:], in_=h2p[:], func=Act.Tanh,
                                     bias=b2[:, g:g + 1], scale=1.0)
                nc.tensor.matmul(
                    out=pre[:],
                    lhsT=wgt[:, NG * P + g * NSLOT:NG * P + (g + 1) * NSLOT],
                    rhs=h2[:], start=(g == 0), stop=(g == NG - 1))

            # ---- phase B: records ----
            term = wpool.tile([NSLOT, KT], f32, tag="term")
            nc.vector.tensor_mul(out=term[:], in0=pre[:], in1=winm)
            tr = psum.tile([P, 2 * NSLOT], f32, tag="tr")
            nc.tensor.transpose(out=tr[:, 0:NSLOT], in_=term[:, 0:P],
                                identity=id12)
            nc.tensor.transpose(out=tr[:, NSLOT:2 * NSLOT], in_=term[:, P:KT],
                                identity=id12)
            v2r = wpool.tile([P, 2], f32, tag="v2r")
            nc.vector.reduce_sum(
                out=v2r[:], axis=mybir.AxisListType.X,
                in_=tr[:, :].rearrange("p (c s) -> p c s", c=2))
            v2 = wpool.tile([P, 2], f32, tag="v2")
            nc.vector.tensor_add(out=v2[:], in0=v2r[:], in1=c02)
            diff = wpool.tile([P, 1], f32, tag="diff")
            nc.vector.tensor_sub(out=diff[:], in0=v2[:, 1:2], in1=v2[:, 0:1])

            # ---- phase C: y = xs*diff + vlo (xs host-prescaled by isl),
            # two pieces on parallel engines/queues; gpsimd is ~2x slower
            # per element than DVE so it gets the smaller piece.
            SPL = (0, (S * 2 // 3 + 7) // 8 * 8, S)
            for q in range(2):
                sl = slice(SPL[q], SPL[q + 1])
                y = wpool.tile([P, SPL[q + 1] - SPL[q]], bf16, tag=f"y{q}")
                teng = nc.vector if q % 2 == 0 else nc.gpsimd
                teng.tensor_scalar(out=y[:], in0=xp[:, sl], scalar1=diff[:],
                                   scalar2=v2[:, 0:1], op0=Op.mult, op1=Op.add)
                eng = nc.sync if q % 2 == 0 else nc.scalar
                eng.dma_start(out=y_out[:, sl], in_=y[:])

    nc.compile()
    _PROGS[S] = nc
    return nc


# ---------------- host-side input prep ----------------------------------------
def _fold_weights(core, W1, b1, W2, b2, W3, b3):
    means, std, mid, Lb, Rb, bnds = _geometry()
    base = DOM0 + core * DW
    act = [w for w in range(NW) if (Rb[w] > base) and (Lb[w] < base + DW)]
    assert len(act) <= NSLOT, f"core {core}: {len(act)} active windows"
    sc1 = np.zeros((P, NG), np.float32)
    bi1 = np.zeros((P, NG), np.float32)
    w2blk = np.zeros((P, P * NG), np.float32)
    w3f = np.zeros((P, NSLOT * NG), np.float32)
    b2c = np.zeros((P, NG), np.float32)
    b3c = np.zeros(NSLOT, np.float32)
    for slot, w in enumerate(act):
        g, s = divmod(slot, 4)
        rows = slice(32 * s, 32 * s + 32)
        w1r = W1[w, 0, :].astype(np.float64)
        sc1[rows, g] = (w1r / std[w]).astype(np.float32)
        bi1[rows, g] = (b1[w] - w1r * means[w] / std[w]).astype(np.float32)
        w2blk[rows, g * P + 32 * s: g * P + 32 * s + 32] = W2[w]
        w3f[rows, g * NSLOT + slot] = W3[w, :, 0]
        b2c[rows, g] = b2[w]
        b3c[slot] = b3[w, 0]
    return sc1, bi1, w2blk, w3f, b2c, b3c, act


def _prep_in_maps(inputs, S):
    x = np.asarray(inputs["x"], np.float32)
    W1 = np.asarray(inputs["W1"], np.float32)
    b1 = np.asarray(inputs["b1"], np.float32)
    W2 = np.asarray(inputs["W2"], np.float32)
    b2 = np.asarray(inputs["b2"], np.float32)
    W3 = np.asarray(inputs["W3"], np.float32)
    b3 = np.asarray(inputs["b3"], np.float32)
    means, std, mid, Lb, Rb, bnds = _geometry()
    cores, glo_lo, glo_slot, los_global, isl_global = _slot_tables()

    idx = np.searchsorted(glo_lo, x, side="right") - 1
    gs = glo_slot[idx]
    cnt = np.bincount(gs, minlength=NCORES * P)
    maxcnt = int(cnt.max())
    if maxcnt > S:
        raise OverflowError(maxcnt)
    order = np.argsort(gs, kind="stable")
    starts = np.concatenate(([0], np.cumsum(cnt)))
    rank = np.arange(len(x)) - starts[gs[order]]
    slotflat = gs[order] * S + rank
    xpad = np.zeros(NCORES * P * S, np.float32)
    go = gs[order]
    xpad[slotflat] = (x[order].astype(np.float64)
                      - los_global[go]) * isl_global[go]
    xpad = xpad.astype(BF16).reshape(NCORES, P, S)

    in_maps = []
    for core in range(NCORES):
        ct = cores[core]
        sc1, bi1, w2blk, w3f, b2c, b3c, act = _fold_weights(
            core, W1, b1, W2, b2, W3, b3)
        kvals = np.concatenate([ct["los"], ct["his"]])       # [KT]
        # win * mask * tanh(x) at every knot is input-independent: fold it
        # into one host table so no window math runs on device.
        k64 = kvals.astype(np.float64)
        winm = np.zeros((NSLOT, KT), np.float32)
        for slot, w in enumerate(act):
            lbv = np.nextafter(Lb[w], -np.inf)
            mask = (kvals > lbv) & (kvals < Rb[w])
            win = _win64(mid[w], mid[w + 1], k64)
            winm[slot] = (mask * win * np.tanh(k64)).astype(np.float32)
        c0 = winm.T @ b3c                                    # [KT]
        # kn3[p, g*KT+k] = sc1[p,g]*knot[k] + bi1[p,g]  (fp64 -> fp16)
        kn3 = (sc1.astype(np.float64).T[:, :, None]
               * kvals.astype(np.float64)[None, None, :]
               + bi1.astype(np.float64).T[:, :, None])      # [NG, P, KT]
        kn3 = kn3.transpose(1, 0, 2).reshape(P, NG * KT).astype(np.float16)
        pk5 = np.zeros((P, 5), np.float32)
        pk5[:, 0:NG] = b2c
        pk5[:, NG] = c0[0:P]
        pk5[:, NG + 1] = c0[P:KT]
        wgt = np.concatenate([w2blk, w3f], axis=1).astype(BF16)
        pk12 = np.concatenate(
            [winm, np.eye(NSLOT, dtype=np.float32)], axis=1)
        in_maps.append({
            "x_pts": xpad[core],
            "kn3": kn3,
            "pk5": pk5,
            "wgt": wgt,
            "pk12": pk12,
        })
    return in_maps, order, slotflat


def _unpack(results, order, slotflat, n_total):
    allys = np.concatenate(
        [np.asarray(r["y_out"]).astype(np.float32).reshape(-1)
         for r in results])
    out = np.empty(n_total, np.float32)
    out[order] = allys[slotflat]
    return out


def kernel(**inputs) -> np.ndarray:
    from concourse.bass_utils import run_bass_kernel_spmd

    S = S_DEFAULT
    while True:
        try:
            in_maps, order, slotflat = _prep_in_maps(inputs, S)
            break
        except OverflowError as e:
            S = ((int(e.args[0]) + 17) // 8) * 8   # headroom, multiple of 8
    nc = _build_program(S)
    res = run_bass_kernel_spmd(nc, in_maps, list(range(NCORES)))
    return _unpack(res.results, order, slotflat, len(np.asarray(inputs["x"])))
